# revision 2
# baseline (speedup 1.0000x reference)
"""Multi-head attention Trainium2 kernel (8 NeuronCores, SPMD).

Problem: B=2, S=2048, D=1024, H=16 heads, d_k=64.
Sharding: 2 batches x 4 head-groups -> 8 cores. Core c handles batch c//4,
heads [4*(c%4), 4*(c%4)+4). Each core computes its 4 heads' Q/K/V
projections, attention, and a partial output projection (row-parallel Wo);
the host sums the 4 partials per batch (the "all-reduce" done on host).

On-device layout is feature-major ("transposed"): activations live as
[d, tokens] so that
  - projections are natural matmuls (lhsT = W.T tiles, rhs = x.T tiles),
  - scores are computed directly as S.T [k_seq, q] (k on partitions),
  - softmax sum over k (partition dim) falls out of the P@V matmul by
    augmenting V with a ones column (row 64 of the PV psum = denominators).
Biases are folded in by augmenting x.T / W.T with a constant-one row
(padded contraction dim 1024 -> 1152 = 9*128). The 1/sqrt(d_k) scale is
folded into Wq/bq on the host.
"""

import os
import threading
from contextlib import ExitStack

import numpy as np

import concourse.bass as bass
import concourse.tile as tile
from concourse import bacc, mybir
from concourse.bass_utils import run_bass_kernel_spmd

F32 = mybir.dt.float32
AF = mybir.ActivationFunctionType

B = 2
S = 2048
D = 1024
H = 16
DK = 64
N_CORES = 8
HG = 4  # heads per core
EW = HG * DK  # 256 features per core
DP = D + 128  # padded contraction (bias row + zero pad): 9 * 128
DT = DP // 128  # 9 contraction tiles
QT = 4  # q tiles of 512
QTS = S // QT  # 512
KT = S // 128  # 16 k-seq tiles of 128
ET = D // 128  # 8 output-feature tiles


def build_program():
    """Build + compile the (single, SPMD) Bass program. Returns nc."""
    nc = bacc.Bacc("TRN2", target_bir_lowering=False, debug=False,
                   num_devices=N_CORES)

    xq = nc.dram_tensor("xq", [DP, S], F32, kind="ExternalInput").ap()
    xk = nc.dram_tensor("xk", [DP, S], F32, kind="ExternalInput").ap()
    xv = nc.dram_tensor("xv", [DP, S], F32, kind="ExternalInput").ap()
    wq = nc.dram_tensor("wq", [DP, EW], F32, kind="ExternalInput").ap()
    wk = nc.dram_tensor("wk", [DP, EW], F32, kind="ExternalInput").ap()
    wv = nc.dram_tensor("wv", [DP, EW], F32, kind="ExternalInput").ap()
    wo = nc.dram_tensor("wo", [DK, HG, D], F32, kind="ExternalInput").ap()
    outT = nc.dram_tensor("outT", [D, S], F32, kind="ExternalOutput").ap()

    xq_v = xq.rearrange("(dt p) n -> p dt n", p=128)
    xk_v = xk.rearrange("(dt p) n -> p dt n", p=128)
    xv_v = xv.rearrange("(dt p) n -> p dt n", p=128)
    wq_v = wq.rearrange("(dt p) m -> p dt m", p=128)
    wk_v = wk.rearrange("(dt p) m -> p dt m", p=128)
    wv_v = wv.rearrange("(dt p) m -> p dt m", p=128)
    outT_v = outT.rearrange("(et p) n -> p et n", p=128)

    with tile.TileContext(nc) as tc:
        with ExitStack() as ctx:
            # pools alive for the whole kernel
            persist = ctx.enter_context(tc.tile_pool(name="persist", bufs=1))
            wo_sb = persist.tile([DK, HG, D], F32, tag="wo_sb")
            OA = persist.tile([128, HG, S], F32, tag="OA")  # rows 0:64 O.T, row 64 = l
            nc.sync.dma_start(wo_sb[:], wo[:])

            with ExitStack() as actx:
                # pools alive through proj + attention
                apers = actx.enter_context(tc.tile_pool(name="apers", bufs=1))
                QTi = apers.tile([128, 2, S], F32, tag="QT")  # pair-stacked Q.T
                KTi = apers.tile([128, 2, S], F32, tag="KT")
                VA = apers.tile([128, KT, HG, DK + 1], F32, tag="VA")
                nc.gpsimd.memset(VA[:, :, :, DK:DK + 1], 1.0)

                # ---- projections ----
                with ExitStack() as pctx:
                    wpool = pctx.enter_context(tc.tile_pool(name="wts", bufs=1))
                    xpool = pctx.enter_context(tc.tile_pool(name="xin", bufs=2))
                    pps = pctx.enter_context(
                        tc.tile_pool(name="pps", bufs=3, space="PSUM"))

                    wq_sb = wpool.tile([128, DT, EW], F32, tag="wq_sb")
                    wk_sb = wpool.tile([128, DT, EW], F32, tag="wk_sb")
                    wv_sb = wpool.tile([128, DT, EW], F32, tag="wv_sb")
                    nc.sync.dma_start(wq_sb[:], wq_v[:])
                    nc.sync.dma_start(wk_sb[:], wk_v[:])
                    nc.sync.dma_start(wv_sb[:], wv_v[:])

                    # Q and K -> transposed layout [feat128(pair-stacked), seq]
                    for qt in range(QT):
                        qs = slice(qt * QTS, (qt + 1) * QTS)
                        for (x_v, w_sb, dst) in ((xq_v, wq_sb, QTi),
                                                 (xk_v, wk_sb, KTi)):
                            xt = xpool.tile([128, DT, QTS], F32, tag="xt")
                            nc.sync.dma_start(xt[:], x_v[:, :, qs])
                            for p in range(2):
                                ps = pps.tile([128, QTS], F32, tag="pqk")
                                for dt in range(DT):
                                    nc.tensor.matmul(
                                        ps[:],
                                        w_sb[:, dt, p * 128:(p + 1) * 128],
                                        xt[:, dt, :],
                                        start=(dt == 0), stop=(dt == DT - 1))
                                nc.vector.tensor_copy(dst[:, p, qs], ps[:])

                    # V -> natural layout [seq128, feat] into VA
                    for st in range(QT):
                        ss_ = slice(st * QTS, (st + 1) * QTS)
                        xt = xpool.tile([128, DT, QTS], F32, tag="xt")
                        nc.sync.dma_start(xt[:], xv_v[:, :, ss_])
                        for ss in range(4):
                            kti = st * 4 + ss
                            ps = pps.tile([128, HG, DK], F32, tag="pv")
                            for dt in range(DT):
                                nc.tensor.matmul(
                                    ps[:],
                                    xt[:, dt, ss * 128:(ss + 1) * 128],
                                    wv_sb[:, dt, :],
                                    start=(dt == 0), stop=(dt == DT - 1))
                            nc.vector.tensor_copy(VA[:, kti, :, 0:DK], ps[:])

                # ---- attention ----
                with ExitStack() as atx:
                    spool = atx.enter_context(
                        tc.tile_pool(name="spsum", bufs=2, space="PSUM"))
                    opool = atx.enter_context(
                        tc.tile_pool(name="opsum", bufs=4, space="PSUM"))
                    ptpool = atx.enter_context(tc.tile_pool(name="pt", bufs=3))

                    for qt in range(QT):
                        qs = slice(qt * QTS, (qt + 1) * QTS)
                        for p in range(2):
                            po = [opool.tile([128, QTS], F32, tag="o",
                                             name=f"po{i}")
                                  for i in range(2)]
                            for kt in range(KT):
                                ks = slice(kt * 128, (kt + 1) * 128)
                                ps_s = spool.tile([128, 2, QTS], F32, tag="s")
                                for hh in range(2):
                                    r0 = 64 * hh
                                    nc.tensor.matmul(
                                        ps_s[:, hh, :],
                                        KTi[r0:r0 + 64, p, ks],
                                        QTi[r0:r0 + 64, p, qs],
                                        start=True, stop=True)
                                pt_t = ptpool.tile([128, 2, QTS], F32, tag="pt")
                                nc.scalar.activation(pt_t[:], ps_s[:], AF.Exp)
                                for hh in range(2):
                                    h = 2 * p + hh
                                    nc.tensor.matmul(
                                        po[hh][0:DK + 1, :],
                                        VA[:, kt, h, :],
                                        pt_t[:, hh, :],
                                        start=(kt == 0), stop=(kt == KT - 1))
                            for hh in range(2):
                                h = 2 * p + hh
                                nc.vector.tensor_copy(
                                    OA[0:DK + 1, h, qs], po[hh][0:DK + 1, :])

            # ---- normalize + output projection ----
            with ExitStack() as octx:
                npool = octx.enter_context(tc.tile_pool(name="norm", bufs=1))
                apool = octx.enter_context(tc.tile_pool(name="atile", bufs=6))
                obuf = octx.enter_context(tc.tile_pool(name="obuf", bufs=3))
                opsum = octx.enter_context(
                    tc.tile_pool(name="outpsum", bufs=2, space="PSUM"))

                r0t = npool.tile([1, HG, S], F32, tag="r0")
                r_bc = npool.tile([DK, HG, S], F32, tag="rbc")
                # move the denominators (partition 64) down to partition 0
                nc.sync.dma_start(r0t[0:1, :, :], OA[DK:DK + 1, :, :])
                nc.vector.reciprocal(r0t[0:1, :, :], r0t[0:1, :, :])
                nc.gpsimd.partition_broadcast(r_bc[:], r0t[0:1, :, :])

                for qt in range(QT):
                    qs = slice(qt * QTS, (qt + 1) * QTS)
                    ats = []
                    for h in range(HG):
                        at = apool.tile([DK, QTS], F32, tag="at")
                        nc.vector.tensor_mul(
                            at[:], OA[0:DK, h, qs], r_bc[:, h, qs])
                        ats.append(at)
                    for et in range(ET):
                        ps = opsum.tile([128, QTS], F32, tag="po")
                        for h in range(HG):
                            nc.tensor.matmul(
                                ps[:],
                                wo_sb[:, h, et * 128:(et + 1) * 128],
                                ats[h][:],
                                start=(h == 0), stop=(h == HG - 1))
                        ot = obuf.tile([128, QTS], F32, tag="ot")
                        nc.scalar.activation(ot[:], ps[:], AF.Copy)
                        nc.sync.dma_start(outT_v[:, et, qs], ot[:])

    nc.compile()
    return nc


_CACHE = {}
_CACHE_LOCK = threading.Lock()


def _get_program():
    with _CACHE_LOCK:
        if "nc" not in _CACHE:
            _CACHE["nc"] = build_program()
        return _CACHE["nc"]


def _prep_inputs(q, k, v, Wq, bq, Wk, bk, Wv, bv, Wo, bo):
    """Build the 8 per-core input maps (all float32 numpy)."""
    scale = 1.0 / np.sqrt(DK)

    def aug_x(x_b):  # [S, D] -> [DP, S]
        out = np.zeros((DP, S), np.float32)
        out[:D] = x_b.T
        out[D] = 1.0
        return np.ascontiguousarray(out)

    def aug_w(W, b, g, sc=1.0):  # rows slice of W -> [DP, EW]
        sl = slice(g * EW, (g + 1) * EW)
        out = np.zeros((DP, EW), np.float32)
        out[:D] = W[sl].T * sc
        out[D] = b[sl] * sc
        return np.ascontiguousarray(out)

    xs = []
    for b_i in range(B):
        xs.append((aug_x(q[b_i]), aug_x(k[b_i]), aug_x(v[b_i])))

    in_maps = []
    for c in range(N_CORES):
        b_i, g = divmod(c, HG)
        wo_c = Wo[:, g * EW:(g + 1) * EW].T  # [EW, D]
        wo_c = np.ascontiguousarray(
            wo_c.reshape(HG, DK, D).transpose(1, 0, 2))  # [DK, HG, D]
        in_maps.append({
            "xq": xs[b_i][0], "xk": xs[b_i][1], "xv": xs[b_i][2],
            "wq": aug_w(Wq, bq, g, scale),
            "wk": aug_w(Wk, bk, g),
            "wv": aug_w(Wv, bv, g),
            "wo": wo_c,
        })
    return in_maps


def kernel(q, k, v, Wq, bq, Wk, bk, Wv, bv, Wo, bo):
    q = np.asarray(q, np.float32)
    k = np.asarray(k, np.float32)
    v = np.asarray(v, np.float32)
    in_maps = _prep_inputs(q, k, v,
                           np.asarray(Wq, np.float32), np.asarray(bq, np.float32),
                           np.asarray(Wk, np.float32), np.asarray(bk, np.float32),
                           np.asarray(Wv, np.float32), np.asarray(bv, np.float32),
                           np.asarray(Wo, np.float32), np.asarray(bo, np.float32))
    nc = _get_program()
    res = run_bass_kernel_spmd(nc, in_maps, core_ids=list(range(N_CORES)))
    out = np.zeros((B, S, D), np.float32)
    for c in range(N_CORES):
        b_i = c // HG
        out[b_i] += res.results[c]["outT"].T
    out += np.asarray(bo, np.float32)
    return out


# revision 4
# speedup vs baseline: 2.2190x; 2.2190x over previous
"""Multi-head attention Trainium2 kernel (8 NeuronCores, SPMD).

Problem: B=2, S=2048, D=1024, H=16 heads, d_k=64.
Sharding: 2 batches x 4 head-groups -> 8 cores. Core c handles batch c//4,
heads [4*(c%4), 4*(c%4)+4). Each core computes its 4 heads' Q/K/V
projections, attention, and a partial output projection (row-parallel Wo);
the host sums the 4 partials per batch (the "all-reduce" done on host).

On-device layout is feature-major ("transposed"): activations live as
[d, tokens] so that
  - projections are natural matmuls (lhsT = W.T tiles, rhs = x.T tiles),
  - scores are computed directly as S.T [k_seq, q] (k on partitions),
  - softmax sum over k (partition dim) falls out of the P@V matmul by
    augmenting V with a ones column (row 64 of the PV psum = denominators).
Biases are folded in by augmenting x.T / W.T with a constant-one row
(padded contraction dim 1024 -> 1152 = 9*128). The 1/sqrt(d_k) scale is
folded into Wq/bq on the host.
"""

import os
import threading
from contextlib import ExitStack

import numpy as np

import concourse.bass as bass
import concourse.tile as tile
from concourse import bacc, mybir
from concourse.bass_utils import run_bass_kernel_spmd

F32 = mybir.dt.float32
F32R = mybir.dt.float32r
AF = mybir.ActivationFunctionType

B = 2
S = 2048
D = 1024
H = 16
DK = 64
N_CORES = 8
HG = 4  # heads per core
EW = HG * DK  # 256 features per core
DP = D + 128  # padded contraction (bias row + zero pad): 9 * 128
DT = DP // 128  # 9 contraction tiles
QT = 4  # q tiles of 512
QTS = S // QT  # 512
KT = S // 128  # 16 k-seq tiles of 128
ET = D // 128  # 8 output-feature tiles


def build_program():
    """Build + compile the (single, SPMD) Bass program. Returns nc."""
    nc = bacc.Bacc("TRN2", target_bir_lowering=False, debug=False,
                   num_devices=N_CORES)

    xq = nc.dram_tensor("xq", [DP, S], F32R, kind="ExternalInput").ap()
    xk = nc.dram_tensor("xk", [DP, S], F32R, kind="ExternalInput").ap()
    xv = nc.dram_tensor("xv", [DP, S], F32R, kind="ExternalInput").ap()
    wq = nc.dram_tensor("wq", [DP, EW], F32R, kind="ExternalInput").ap()
    wk = nc.dram_tensor("wk", [DP, EW], F32R, kind="ExternalInput").ap()
    wv = nc.dram_tensor("wv", [DP, EW], F32R, kind="ExternalInput").ap()
    wo = nc.dram_tensor("wo", [DK, HG, D], F32R, kind="ExternalInput").ap()
    outT = nc.dram_tensor("outT", [D, S], F32, kind="ExternalOutput").ap()

    xq_v = xq.rearrange("(dt p) n -> p dt n", p=128)
    xk_v = xk.rearrange("(dt p) n -> p dt n", p=128)
    xv_v = xv.rearrange("(dt p) n -> p dt n", p=128)
    wq_v = wq.rearrange("(dt p) m -> p dt m", p=128)
    wk_v = wk.rearrange("(dt p) m -> p dt m", p=128)
    wv_v = wv.rearrange("(dt p) m -> p dt m", p=128)
    outT_v = outT.rearrange("(et p) n -> p et n", p=128)

    with tile.TileContext(nc) as tc:
        with ExitStack() as ctx:
            # pools alive for the whole kernel
            persist = ctx.enter_context(tc.tile_pool(name="persist", bufs=1))
            wo_sb = persist.tile([DK, HG, D], F32R, tag="wo_sb")
            OA = persist.tile([128, HG, S], F32, tag="OA")  # rows 0:64 O.T, row 64 = l
            nc.sync.dma_start(wo_sb[:], wo[:])

            with ExitStack() as actx:
                # pools alive through proj + attention
                apers = actx.enter_context(tc.tile_pool(name="apers", bufs=1))
                QTi = apers.tile([128, 2, S], F32R, tag="QT")  # pair-stacked Q.T
                KTi = apers.tile([128, 2, S], F32R, tag="KT")
                VA = apers.tile([128, KT, HG, DK + 1], F32R, tag="VA")
                nc.gpsimd.memset(VA[:, :, :, DK:DK + 1].bitcast(F32), 1.0)

                # ---- projections ----
                with ExitStack() as pctx:
                    wpool = pctx.enter_context(tc.tile_pool(name="wts", bufs=1))
                    xpool = pctx.enter_context(tc.tile_pool(name="xin", bufs=2))
                    pps = pctx.enter_context(
                        tc.tile_pool(name="pps", bufs=3, space="PSUM"))

                    wq_sb = wpool.tile([128, DT, EW], F32R, tag="wq_sb")
                    wk_sb = wpool.tile([128, DT, EW], F32R, tag="wk_sb")
                    wv_sb = wpool.tile([128, DT, EW], F32R, tag="wv_sb")
                    nc.sync.dma_start(wq_sb[:], wq_v[:])
                    nc.sync.dma_start(wk_sb[:], wk_v[:])
                    nc.sync.dma_start(wv_sb[:], wv_v[:])

                    # Q and K -> transposed layout [feat128(pair-stacked), seq]
                    for qt in range(QT):
                        qs = slice(qt * QTS, (qt + 1) * QTS)
                        for (x_v, w_sb, dst) in ((xq_v, wq_sb, QTi),
                                                 (xk_v, wk_sb, KTi)):
                            xt = xpool.tile([128, DT, QTS], F32R, tag="xt")
                            nc.sync.dma_start(xt[:], x_v[:, :, qs])
                            for p in range(2):
                                ps = pps.tile([128, QTS], F32, tag="pqk")
                                for dt in range(DT):
                                    nc.tensor.matmul(
                                        ps[:],
                                        w_sb[:, dt, p * 128:(p + 1) * 128],
                                        xt[:, dt, :],
                                        start=(dt == 0), stop=(dt == DT - 1))
                                nc.vector.tensor_copy(dst[:, p, qs], ps[:])

                    # V -> natural layout [seq128, feat] into VA
                    for st in range(QT):
                        ss_ = slice(st * QTS, (st + 1) * QTS)
                        xt = xpool.tile([128, DT, QTS], F32R, tag="xt")
                        nc.sync.dma_start(xt[:], xv_v[:, :, ss_])
                        for ss in range(4):
                            kti = st * 4 + ss
                            ps = pps.tile([128, HG, DK], F32, tag="pv")
                            for dt in range(DT):
                                nc.tensor.matmul(
                                    ps[:],
                                    xt[:, dt, ss * 128:(ss + 1) * 128],
                                    wv_sb[:, dt, :],
                                    start=(dt == 0), stop=(dt == DT - 1))
                            nc.vector.tensor_copy(VA[:, kti, :, 0:DK], ps[:])

                # ---- attention ----
                with ExitStack() as atx:
                    spool = atx.enter_context(
                        tc.tile_pool(name="spsum", bufs=2, space="PSUM"))
                    opool = atx.enter_context(
                        tc.tile_pool(name="opsum", bufs=4, space="PSUM"))
                    ptpool = atx.enter_context(tc.tile_pool(name="pt", bufs=3))

                    for qt in range(QT):
                        qs = slice(qt * QTS, (qt + 1) * QTS)
                        for p in range(2):
                            po = [opool.tile([128, QTS], F32, tag="o",
                                             name=f"po{i}")
                                  for i in range(2)]
                            for kt in range(KT):
                                ks = slice(kt * 128, (kt + 1) * 128)
                                ps_s = spool.tile([128, 2, QTS], F32, tag="s")
                                for hh in range(2):
                                    r0 = 64 * hh
                                    nc.tensor.matmul(
                                        ps_s[:, hh, :],
                                        KTi[r0:r0 + 64, p, ks],
                                        QTi[r0:r0 + 64, p, qs],
                                        start=True, stop=True)
                                pt_t = ptpool.tile([128, 2, QTS], F32R, tag="pt")
                                nc.scalar.activation(pt_t[:], ps_s[:], AF.Exp)
                                for hh in range(2):
                                    h = 2 * p + hh
                                    nc.tensor.matmul(
                                        po[hh][0:DK + 1, :],
                                        VA[:, kt, h, :],
                                        pt_t[:, hh, :],
                                        start=(kt == 0), stop=(kt == KT - 1))
                            for hh in range(2):
                                h = 2 * p + hh
                                nc.vector.tensor_copy(
                                    OA[0:DK + 1, h, qs], po[hh][0:DK + 1, :])

            # ---- normalize + output projection ----
            with ExitStack() as octx:
                npool = octx.enter_context(tc.tile_pool(name="norm", bufs=1))
                apool = octx.enter_context(tc.tile_pool(name="atile", bufs=6))
                obuf = octx.enter_context(tc.tile_pool(name="obuf", bufs=3))
                opsum = octx.enter_context(
                    tc.tile_pool(name="outpsum", bufs=2, space="PSUM"))

                r0t = npool.tile([1, HG, S], F32, tag="r0")
                r_bc = npool.tile([DK, HG, S], F32, tag="rbc")
                # move the denominators (partition 64) down to partition 0
                nc.sync.dma_start(r0t[0:1, :, :], OA[DK:DK + 1, :, :])
                nc.vector.reciprocal(r0t[0:1, :, :], r0t[0:1, :, :])
                nc.gpsimd.partition_broadcast(r_bc[:], r0t[0:1, :, :])

                for qt in range(QT):
                    qs = slice(qt * QTS, (qt + 1) * QTS)
                    ats = []
                    for h in range(HG):
                        at = apool.tile([DK, QTS], F32R, tag="at")
                        nc.vector.tensor_mul(
                            at[:], OA[0:DK, h, qs], r_bc[:, h, qs])
                        ats.append(at)
                    for et in range(ET):
                        ps = opsum.tile([128, QTS], F32, tag="po")
                        for h in range(HG):
                            nc.tensor.matmul(
                                ps[:],
                                wo_sb[:, h, et * 128:(et + 1) * 128],
                                ats[h][:],
                                start=(h == 0), stop=(h == HG - 1))
                        ot = obuf.tile([128, QTS], F32, tag="ot")
                        nc.vector.tensor_copy(ot[:], ps[:])
                        nc.sync.dma_start(outT_v[:, et, qs], ot[:])

    nc.compile()
    return nc


_CACHE = {}
_CACHE_LOCK = threading.Lock()


def _get_program():
    with _CACHE_LOCK:
        if "nc" not in _CACHE:
            _CACHE["nc"] = build_program()
        return _CACHE["nc"]


def _prep_inputs(q, k, v, Wq, bq, Wk, bk, Wv, bv, Wo, bo):
    """Build the 8 per-core input maps (all float32 numpy)."""
    scale = 1.0 / np.sqrt(DK)

    def aug_x(x_b):  # [S, D] -> [DP, S]
        out = np.zeros((DP, S), np.float32)
        out[:D] = x_b.T
        out[D] = 1.0
        return np.ascontiguousarray(out)

    def aug_w(W, b, g, sc=1.0):  # rows slice of W -> [DP, EW]
        sl = slice(g * EW, (g + 1) * EW)
        out = np.zeros((DP, EW), np.float32)
        out[:D] = W[sl].T * sc
        out[D] = b[sl] * sc
        return np.ascontiguousarray(out)

    xs = []
    for b_i in range(B):
        xs.append((aug_x(q[b_i]), aug_x(k[b_i]), aug_x(v[b_i])))

    in_maps = []
    for c in range(N_CORES):
        b_i, g = divmod(c, HG)
        wo_c = Wo[:, g * EW:(g + 1) * EW].T  # [EW, D]
        wo_c = np.ascontiguousarray(
            wo_c.reshape(HG, DK, D).transpose(1, 0, 2))  # [DK, HG, D]
        in_maps.append({
            "xq": xs[b_i][0], "xk": xs[b_i][1], "xv": xs[b_i][2],
            "wq": aug_w(Wq, bq, g, scale),
            "wk": aug_w(Wk, bk, g),
            "wv": aug_w(Wv, bv, g),
            "wo": wo_c,
        })
    return in_maps


def kernel(q, k, v, Wq, bq, Wk, bk, Wv, bv, Wo, bo):
    q = np.asarray(q, np.float32)
    k = np.asarray(k, np.float32)
    v = np.asarray(v, np.float32)
    in_maps = _prep_inputs(q, k, v,
                           np.asarray(Wq, np.float32), np.asarray(bq, np.float32),
                           np.asarray(Wk, np.float32), np.asarray(bk, np.float32),
                           np.asarray(Wv, np.float32), np.asarray(bv, np.float32),
                           np.asarray(Wo, np.float32), np.asarray(bo, np.float32))
    nc = _get_program()
    res = run_bass_kernel_spmd(nc, in_maps, core_ids=list(range(N_CORES)))
    out = np.zeros((B, S, D), np.float32)
    for c in range(N_CORES):
        b_i = c // HG
        out[b_i] += res.results[c]["outT"].T
    out += np.asarray(bo, np.float32)
    return out


# revision 5
# speedup vs baseline: 2.4269x; 1.0937x over previous
"""Multi-head attention Trainium2 kernel (8 NeuronCores, SPMD).

Problem: B=2, S=2048, D=1024, H=16 heads, d_k=64.
Sharding: 2 batches x 4 head-groups -> 8 cores. Core c handles batch c//4,
heads [4*(c%4), 4*(c%4)+4). Each core computes its 4 heads' Q/K/V
projections, attention, and a partial output projection (row-parallel Wo);
the host sums the 4 partials per batch (the "all-reduce" done on host).

On-device layout is feature-major ("transposed"): activations live as
[d, tokens] so that
  - projections are natural matmuls (lhsT = W.T tiles, rhs = x.T tiles),
  - scores are computed directly as S.T [k_seq, q] (k on partitions),
  - softmax sum over k (partition dim) falls out of the P@V matmul by
    augmenting V with a ones column (row 64 of the PV psum = denominators).
Biases are folded in by augmenting x.T / W.T with a constant-one row
(padded contraction dim 1024 -> 1152 = 9*128). The 1/sqrt(d_k) scale is
folded into Wq/bq on the host. Matmul operands use float32r (full-rate
PE fp32); accumulation stays fp32.

Phase order: K proj, V proj, then per q-tile {Q proj, attention, softmax
normalize, output projection} so DMA/PE/ACT/DVE phases overlap.
"""

import threading
from contextlib import ExitStack

import numpy as np

import concourse.bass as bass
import concourse.tile as tile
from concourse import bacc, mybir
from concourse.bass_utils import run_bass_kernel_spmd

F32 = mybir.dt.float32
F32R = mybir.dt.float32r
AF = mybir.ActivationFunctionType

B = 2
S = 2048
D = 1024
H = 16
DK = 64
N_CORES = 8
HG = 4  # heads per core
EW = HG * DK  # 256 features per core
DP = D + 128  # padded contraction (bias row + zero pad): 9 * 128
DT = DP // 128  # 9 contraction tiles
QT = 4  # q tiles of 512
QTS = S // QT  # 512
KT = S // 128  # 16 k-seq tiles of 128
ET = D // 128  # 8 output-feature tiles


def build_program():
    """Build + compile the (single, SPMD) Bass program. Returns nc."""
    nc = bacc.Bacc("TRN2", target_bir_lowering=False, debug=False,
                   num_devices=N_CORES)

    xq = nc.dram_tensor("xq", [DP, S], F32R, kind="ExternalInput").ap()
    xk = nc.dram_tensor("xk", [DP, S], F32R, kind="ExternalInput").ap()
    xv = nc.dram_tensor("xv", [DP, S], F32R, kind="ExternalInput").ap()
    wq = nc.dram_tensor("wq", [DP, EW], F32R, kind="ExternalInput").ap()
    wk = nc.dram_tensor("wk", [DP, EW], F32R, kind="ExternalInput").ap()
    wv = nc.dram_tensor("wv", [DP, EW], F32R, kind="ExternalInput").ap()
    wo = nc.dram_tensor("wo", [DK, HG, D], F32R, kind="ExternalInput").ap()
    outT = nc.dram_tensor("outT", [D, S], F32, kind="ExternalOutput").ap()

    xq_v = xq.rearrange("(dt p) n -> p dt n", p=128)
    xk_v = xk.rearrange("(dt p) n -> p dt n", p=128)
    xv_v = xv.rearrange("(dt p) n -> p dt n", p=128)
    wq_v = wq.rearrange("(dt p) m -> p dt m", p=128)
    wk_v = wk.rearrange("(dt p) m -> p dt m", p=128)
    wv_v = wv.rearrange("(dt p) m -> p dt m", p=128)
    outT_v = outT.rearrange("(et p) n -> p et n", p=128)

    with tile.TileContext(nc) as tc, ExitStack() as ctx:
        persist = ctx.enter_context(tc.tile_pool(name="persist", bufs=1))
        wo_sb = persist.tile([DK, HG, D], F32R, tag="wo_sb")
        QTi = persist.tile([128, 2, S], F32R, tag="QT")  # pair-stacked Q.T
        KTi = persist.tile([128, 2, S], F32R, tag="KT")
        VA = persist.tile([128, KT, HG, DK + 1], F32R, tag="VA")
        wpool = ctx.enter_context(tc.tile_pool(name="wts", bufs=1))
        wq_sb = wpool.tile([128, DT, EW], F32R, tag="wq_sb")
        wk_sb = wpool.tile([128, DT, EW], F32R, tag="wk_sb")
        wv_sb = wpool.tile([128, DT, EW], F32R, tag="wv_sb")
        nc.sync.dma_start(wk_sb[:], wk_v[:])
        nc.sync.dma_start(wv_sb[:], wv_v[:])
        nc.sync.dma_start(wq_sb[:], wq_v[:])
        nc.sync.dma_start(wo_sb[:], wo[:])
        nc.gpsimd.memset(VA[:, :, :, DK:DK + 1].bitcast(F32), 1.0)

        xpool = ctx.enter_context(tc.tile_pool(name="xin", bufs=2))
        # single PSUM pool, statically 8 banks:
        #   tag "s": [128, 2, 512] (2 banks) x 2 bufs = 4 banks
        #   tag "o": [128, 512]    (1 bank)  x 4 bufs = 4 banks
        psp = ctx.enter_context(tc.tile_pool(name="psp", bufs=1, space="PSUM"))
        ptpool = ctx.enter_context(tc.tile_pool(name="pt", bufs=3))
        oapool = ctx.enter_context(tc.tile_pool(name="oa", bufs=2))
        rpool = ctx.enter_context(tc.tile_pool(name="rr", bufs=2))
        apool = ctx.enter_context(tc.tile_pool(name="atile", bufs=6))
        obuf = ctx.enter_context(tc.tile_pool(name="obuf", bufs=3))

        def otile(name):
            return psp.tile([128, QTS], F32, tag="o", bufs=4, name=name)

        def stile(name):
            return psp.tile([128, 2, QTS], F32, tag="s", bufs=2, name=name)

        # ---- K projection (transposed layout), then V (natural layout) ----
        for qt in range(QT):
            qs = slice(qt * QTS, (qt + 1) * QTS)
            xt = xpool.tile([128, DT, QTS], F32R, tag="xt", name="xkt")
            nc.sync.dma_start(xt[:], xk_v[:, :, qs])
            for p in range(2):
                ps = otile("psk")
                for dt in range(DT):
                    nc.tensor.matmul(
                        ps[:], wk_sb[:, dt, p * 128:(p + 1) * 128],
                        xt[:, dt, :], start=(dt == 0), stop=(dt == DT - 1))
                nc.vector.tensor_copy(KTi[:, p, qs], ps[:])

        for st in range(QT):
            ss_ = slice(st * QTS, (st + 1) * QTS)
            xt = xpool.tile([128, DT, QTS], F32R, tag="xt", name="xvt")
            nc.sync.dma_start(xt[:], xv_v[:, :, ss_])
            for ss in range(4):
                kti = st * 4 + ss
                ps = otile("psv")
                for dt in range(DT):
                    nc.tensor.matmul(
                        ps[:, 0:EW], xt[:, dt, ss * 128:(ss + 1) * 128],
                        wv_sb[:, dt, :], start=(dt == 0), stop=(dt == DT - 1))
                nc.vector.tensor_copy(
                    VA[:, kti, :, 0:DK],
                    ps[:, 0:EW].rearrange("p (h d) -> p h d", h=HG))

        # ---- per q-tile: Q proj -> attention -> normalize -> out proj ----
        for qt in range(QT):
            qs = slice(qt * QTS, (qt + 1) * QTS)

            xt = xpool.tile([128, DT, QTS], F32R, tag="xt", name="xqt")
            nc.sync.dma_start(xt[:], xq_v[:, :, qs])
            for p in range(2):
                ps = otile("psq")
                for dt in range(DT):
                    nc.tensor.matmul(
                        ps[:], wq_sb[:, dt, p * 128:(p + 1) * 128],
                        xt[:, dt, :], start=(dt == 0), stop=(dt == DT - 1))
                nc.vector.tensor_copy(QTi[:, p, qs], ps[:])

            # attention: S.T = K.T-tiles @ Q-tile, exp, P.T @ V_aug
            OA_t = oapool.tile([128, HG, QTS], F32, tag="oa")
            for p in range(2):
                po = [otile(f"po{i}") for i in range(2)]
                for kt in range(KT):
                    ks = slice(kt * 128, (kt + 1) * 128)
                    ps_s = stile("pss")
                    for hh in range(2):
                        r0 = 64 * hh
                        nc.tensor.matmul(
                            ps_s[:, hh, :],
                            KTi[r0:r0 + 64, p, ks],
                            QTi[r0:r0 + 64, p, qs],
                            start=True, stop=True)
                    pt_t = ptpool.tile([128, 2, QTS], F32R, tag="pt")
                    nc.scalar.activation(pt_t[:], ps_s[:], AF.Exp)
                    for hh in range(2):
                        h = 2 * p + hh
                        nc.tensor.matmul(
                            po[hh][0:DK + 1, :], VA[:, kt, h, :],
                            pt_t[:, hh, :],
                            start=(kt == 0), stop=(kt == KT - 1))
                for hh in range(2):
                    h = 2 * p + hh
                    nc.vector.tensor_copy(
                        OA_t[0:DK + 1, h, :], po[hh][0:DK + 1, :])

            # normalize: r = 1/l broadcast down 64 partitions
            r0_t = rpool.tile([1, HG, QTS], F32, tag="r0")
            rb_t = rpool.tile([DK, HG, QTS], F32, tag="rb")
            nc.sync.dma_start(r0_t[0:1, :, :], OA_t[DK:DK + 1, :, :])
            nc.vector.reciprocal(r0_t[0:1, :, :], r0_t[0:1, :, :])
            nc.gpsimd.partition_broadcast(rb_t[:], r0_t[0:1, :, :])

            ats = []
            for h in range(HG):
                at = apool.tile([DK, QTS], F32R, tag="at", name=f"at{h}")
                nc.vector.tensor_mul(at[:], OA_t[0:DK, h, :], rb_t[:, h, :])
                ats.append(at)
            for et in range(ET):
                ps = otile("pso")
                for h in range(HG):
                    nc.tensor.matmul(
                        ps[:], wo_sb[:, h, et * 128:(et + 1) * 128],
                        ats[h][:], start=(h == 0), stop=(h == HG - 1))
                ot = obuf.tile([128, QTS], F32, tag="ot")
                nc.vector.tensor_copy(ot[:], ps[:])
                nc.sync.dma_start(outT_v[:, et, qs], ot[:])

    nc.compile()
    return nc


_CACHE = {}
_CACHE_LOCK = threading.Lock()


def _get_program():
    with _CACHE_LOCK:
        if "nc" not in _CACHE:
            _CACHE["nc"] = build_program()
        return _CACHE["nc"]


def _prep_inputs(q, k, v, Wq, bq, Wk, bk, Wv, bv, Wo, bo):
    """Build the 8 per-core input maps (all float32 numpy)."""
    scale = 1.0 / np.sqrt(DK)

    def aug_x(x_b):  # [S, D] -> [DP, S]
        out = np.zeros((DP, S), np.float32)
        out[:D] = x_b.T
        out[D] = 1.0
        return np.ascontiguousarray(out)

    def aug_w(W, b, g, sc=1.0):  # rows slice of W -> [DP, EW]
        sl = slice(g * EW, (g + 1) * EW)
        out = np.zeros((DP, EW), np.float32)
        out[:D] = W[sl].T * sc
        out[D] = b[sl] * sc
        return np.ascontiguousarray(out)

    xs = []
    for b_i in range(B):
        xs.append((aug_x(q[b_i]), aug_x(k[b_i]), aug_x(v[b_i])))

    in_maps = []
    for c in range(N_CORES):
        b_i, g = divmod(c, HG)
        wo_c = Wo[:, g * EW:(g + 1) * EW].T  # [EW, D]
        wo_c = np.ascontiguousarray(
            wo_c.reshape(HG, DK, D).transpose(1, 0, 2))  # [DK, HG, D]
        in_maps.append({
            "xq": xs[b_i][0], "xk": xs[b_i][1], "xv": xs[b_i][2],
            "wq": aug_w(Wq, bq, g, scale),
            "wk": aug_w(Wk, bk, g),
            "wv": aug_w(Wv, bv, g),
            "wo": wo_c,
        })
    return in_maps


def kernel(q, k, v, Wq, bq, Wk, bk, Wv, bv, Wo, bo):
    q = np.asarray(q, np.float32)
    k = np.asarray(k, np.float32)
    v = np.asarray(v, np.float32)
    in_maps = _prep_inputs(q, k, v,
                           np.asarray(Wq, np.float32), np.asarray(bq, np.float32),
                           np.asarray(Wk, np.float32), np.asarray(bk, np.float32),
                           np.asarray(Wv, np.float32), np.asarray(bv, np.float32),
                           np.asarray(Wo, np.float32), np.asarray(bo, np.float32))
    nc = _get_program()
    res = run_bass_kernel_spmd(nc, in_maps, core_ids=list(range(N_CORES)))
    out = np.zeros((B, S, D), np.float32)
    for c in range(N_CORES):
        b_i = c // HG
        out[b_i] += res.results[c]["outT"].T
    out += np.asarray(bo, np.float32)
    return out


# revision 6
# speedup vs baseline: 2.9335x; 1.2087x over previous
"""Multi-head attention Trainium2 kernel (8 NeuronCores, SPMD).

Problem: B=2, S=2048, D=1024, H=16 heads, d_k=64.
Sharding: 2 batches x 4 head-groups -> 8 cores. Core c handles batch c//4,
heads [4*(c%4), 4*(c%4)+4). Each core computes its 4 heads' Q/K/V
projections, attention, and a partial output projection (row-parallel Wo);
the host sums the 4 partials per batch (the "all-reduce" done on host).

On-device layout is feature-major ("transposed"): activations live as
[d, tokens] so that
  - projections are natural matmuls (lhsT = W.T tiles, rhs = x.T tiles),
  - scores are computed directly as S.T [k_seq, q] (k on partitions),
  - softmax sum over k (partition dim) falls out of the P@V matmul by
    augmenting V with a ones column (row 64 of the PV psum = denominators).
Biases are folded in by augmenting x.T / W.T with a constant-one row
(padded contraction dim 1024 -> 1152 = 9*128). The 1/sqrt(d_k) scale is
folded into Wq/bq on the host. Matmul operands use float32r (full-rate
PE fp32); accumulation stays fp32.

Phase order: K proj, V proj, then per q-tile {Q proj, attention, softmax
normalize, output projection} so DMA/PE/ACT/DVE phases overlap.
"""

import threading
from contextlib import ExitStack

import numpy as np

import concourse.bass as bass
import concourse.tile as tile
from concourse import bacc, mybir
from concourse.bass_utils import run_bass_kernel_spmd

F32 = mybir.dt.float32
F32R = mybir.dt.float32r
AF = mybir.ActivationFunctionType

B = 2
S = 2048
D = 1024
H = 16
DK = 64
N_CORES = 8
HG = 4  # heads per core
EW = HG * DK  # 256 features per core
DP = D + 128  # padded contraction (bias row + zero pad): 9 * 128
DT = DP // 128  # 9 contraction tiles
QT = 4  # q tiles of 512
QTS = S // QT  # 512
KT = S // 128  # 16 k-seq tiles of 128
ET = D // 128  # 8 output-feature tiles


def build_program():
    """Build + compile the (single, SPMD) Bass program. Returns nc."""
    nc = bacc.Bacc("TRN2", target_bir_lowering=False, debug=False,
                   num_devices=N_CORES)

    xq = nc.dram_tensor("xq", [DP, S], F32R, kind="ExternalInput").ap()
    xk = nc.dram_tensor("xk", [DP, S], F32R, kind="ExternalInput").ap()
    xv = nc.dram_tensor("xv", [DP, S], F32R, kind="ExternalInput").ap()
    wq = nc.dram_tensor("wq", [DP, EW], F32R, kind="ExternalInput").ap()
    wk = nc.dram_tensor("wk", [DP, EW], F32R, kind="ExternalInput").ap()
    wv = nc.dram_tensor("wv", [DP, EW], F32R, kind="ExternalInput").ap()
    wo = nc.dram_tensor("wo", [DK, HG, D], F32R, kind="ExternalInput").ap()
    outT = nc.dram_tensor("outT", [D, S], F32, kind="ExternalOutput").ap()

    xq_v = xq.rearrange("(dt p) n -> p dt n", p=128)
    xk_v = xk.rearrange("(dt p) n -> p dt n", p=128)
    xv_v = xv.rearrange("(dt p) n -> p dt n", p=128)
    wq_v = wq.rearrange("(dt p) m -> p dt m", p=128)
    wk_v = wk.rearrange("(dt p) m -> p dt m", p=128)
    wv_v = wv.rearrange("(dt p) m -> p dt m", p=128)
    outT_v = outT.rearrange("(et p) n -> p et n", p=128)

    with tile.TileContext(nc) as tc, ExitStack() as ctx:
        persist = ctx.enter_context(tc.tile_pool(name="persist", bufs=1))
        wo_sb = persist.tile([DK, HG, D], F32R, tag="wo_sb")
        QTi = persist.tile([128, 2, S], F32R, tag="QT")  # pair-stacked Q.T
        KTi = persist.tile([128, 2, S], F32R, tag="KT")
        VA = persist.tile([128, KT, HG, DK + 1], F32R, tag="VA")
        wpool = ctx.enter_context(tc.tile_pool(name="wts", bufs=1))
        wq_sb = wpool.tile([128, DT, EW], F32R, tag="wq_sb")
        wk_sb = wpool.tile([128, DT, EW], F32R, tag="wk_sb")
        wv_sb = wpool.tile([128, DT, EW], F32R, tag="wv_sb")
        nc.sync.dma_start(wk_sb[:], wk_v[:])
        nc.gpsimd.memset(VA[:, :, :, DK:DK + 1].bitcast(F32), 1.0)

        xpool = ctx.enter_context(tc.tile_pool(name="xin", bufs=2))
        # single PSUM pool, statically 8 banks:
        #   tag "s": [128, 2, 512] (2 banks) x 2 bufs = 4 banks
        #   tag "o": [128, 512]    (1 bank)  x 4 bufs = 4 banks
        psp = ctx.enter_context(tc.tile_pool(name="psp", bufs=1, space="PSUM"))
        ptpool = ctx.enter_context(tc.tile_pool(name="pt", bufs=3))
        oapool = ctx.enter_context(tc.tile_pool(name="oa", bufs=2))
        rpool = ctx.enter_context(tc.tile_pool(name="rr", bufs=2))
        apool = ctx.enter_context(tc.tile_pool(name="atile", bufs=6))
        obuf = ctx.enter_context(tc.tile_pool(name="obuf", bufs=3))

        def otile(name):
            return psp.tile([128, QTS], F32, tag="o", bufs=4, name=name)

        def stile(name):
            return psp.tile([128, 2, QTS], F32, tag="s", bufs=2, name=name)

        # ---- K projection (transposed layout), then V (natural layout) ----
        for qt in range(QT):
            qs = slice(qt * QTS, (qt + 1) * QTS)
            xt = xpool.tile([128, DT, QTS], F32R, tag="xt", name="xkt")
            nc.sync.dma_start(xt[:], xk_v[:, :, qs])
            for p in range(2):
                ps = otile("psk")
                for dt in range(DT):
                    nc.tensor.matmul(
                        ps[:], wk_sb[:, dt, p * 128:(p + 1) * 128],
                        xt[:, dt, :], start=(dt == 0), stop=(dt == DT - 1))
                nc.vector.tensor_copy(KTi[:, p, qs], ps[:])

        nc.sync.dma_start(wq_sb[:], wq_v[:])

        def q_proj(qt):
            qs = slice(qt * QTS, (qt + 1) * QTS)
            xt = xpool.tile([128, DT, QTS], F32R, tag="xt", name="xqt")
            nc.sync.dma_start(xt[:], xq_v[:, :, qs])
            for p in range(2):
                ps = otile("psq")
                for dt in range(DT):
                    nc.tensor.matmul(
                        ps[:], wq_sb[:, dt, p * 128:(p + 1) * 128],
                        xt[:, dt, :], start=(dt == 0), stop=(dt == DT - 1))
                nc.vector.tensor_copy(QTi[:, p, qs], ps[:])

        q_proj(0)
        nc.sync.dma_start(wv_sb[:], wv_v[:])
        for st in range(QT):
            ss_ = slice(st * QTS, (st + 1) * QTS)
            xt = xpool.tile([128, DT, QTS], F32R, tag="xt", name="xvt")
            nc.sync.dma_start(xt[:], xv_v[:, :, ss_])
            for ss in range(4):
                kti = st * 4 + ss
                ps = otile("psv")
                for dt in range(DT):
                    nc.tensor.matmul(
                        ps[:, 0:EW], xt[:, dt, ss * 128:(ss + 1) * 128],
                        wv_sb[:, dt, :], start=(dt == 0), stop=(dt == DT - 1))
                nc.vector.tensor_copy(
                    VA[:, kti, :, 0:DK],
                    ps[:, 0:EW].rearrange("p (h d) -> p h d", h=HG))

        # ---- per q-tile: attention; norm+out-proj pipelined one qt behind ----
        nc.sync.dma_start(wo_sb[:], wo[:])

        def attention(qt):
            qs = slice(qt * QTS, (qt + 1) * QTS)
            OA_t = oapool.tile([128, HG, QTS], F32, tag="oa", name="OA_t")
            for p in range(2):
                po = [otile(f"po{i}") for i in range(2)]
                for kt in range(KT):
                    ks = slice(kt * 128, (kt + 1) * 128)
                    ps_s = stile("pss")
                    for hh in range(2):
                        r0 = 64 * hh
                        nc.tensor.matmul(
                            ps_s[:, hh, :],
                            KTi[r0:r0 + 64, p, ks],
                            QTi[r0:r0 + 64, p, qs],
                            start=True, stop=True)
                    pt_t = ptpool.tile([128, 2, QTS], F32R, tag="pt")
                    nc.scalar.activation(pt_t[:], ps_s[:], AF.Exp)
                    for hh in range(2):
                        h = 2 * p + hh
                        nc.tensor.matmul(
                            po[hh][0:DK + 1, :], VA[:, kt, h, :],
                            pt_t[:, hh, :],
                            start=(kt == 0), stop=(kt == KT - 1))
                for hh in range(2):
                    h = 2 * p + hh
                    nc.vector.tensor_copy(
                        OA_t[0:DK + 1, h, :], po[hh][0:DK + 1, :])
            return OA_t

        def norm_outproj(qt, OA_t):
            qs = slice(qt * QTS, (qt + 1) * QTS)
            r0_t = rpool.tile([1, HG, QTS], F32, tag="r0", name="r0_t")
            rb_t = rpool.tile([DK, HG, QTS], F32, tag="rb", name="rb_t")
            nc.sync.dma_start(r0_t[0:1, :, :], OA_t[DK:DK + 1, :, :])
            nc.vector.reciprocal(r0_t[0:1, :, :], r0_t[0:1, :, :])
            nc.gpsimd.partition_broadcast(rb_t[:], r0_t[0:1, :, :])
            ats = []
            for h in range(HG):
                at = apool.tile([DK, QTS], F32R, tag="at", name=f"at{h}")
                nc.vector.tensor_mul(at[:], OA_t[0:DK, h, :], rb_t[:, h, :])
                ats.append(at)
            for et in range(ET):
                ps = otile("pso")
                for h in range(HG):
                    nc.tensor.matmul(
                        ps[:], wo_sb[:, h, et * 128:(et + 1) * 128],
                        ats[h][:], start=(h == 0), stop=(h == HG - 1))
                ot = obuf.tile([128, QTS], F32, tag="ot")
                nc.vector.tensor_copy(ot[:], ps[:])
                nc.sync.dma_start(outT_v[:, et, qs], ot[:])

        pending = None
        for qt in range(QT):
            if qt + 1 < QT:
                q_proj(qt + 1)
            OA_t = attention(qt)
            if pending is not None:
                norm_outproj(*pending)
            pending = (qt, OA_t)
        norm_outproj(*pending)

    nc.compile()
    return nc


_CACHE = {}
_CACHE_LOCK = threading.Lock()


def _get_program():
    with _CACHE_LOCK:
        if "nc" not in _CACHE:
            _CACHE["nc"] = build_program()
        return _CACHE["nc"]


def _prep_inputs(q, k, v, Wq, bq, Wk, bk, Wv, bv, Wo, bo):
    """Build the 8 per-core input maps (all float32 numpy)."""
    scale = 1.0 / np.sqrt(DK)

    def aug_x(x_b):  # [S, D] -> [DP, S]
        out = np.zeros((DP, S), np.float32)
        out[:D] = x_b.T
        out[D] = 1.0
        return np.ascontiguousarray(out)

    def aug_w(W, b, g, sc=1.0):  # rows slice of W -> [DP, EW]
        sl = slice(g * EW, (g + 1) * EW)
        out = np.zeros((DP, EW), np.float32)
        out[:D] = W[sl].T * sc
        out[D] = b[sl] * sc
        return np.ascontiguousarray(out)

    xs = []
    for b_i in range(B):
        xs.append((aug_x(q[b_i]), aug_x(k[b_i]), aug_x(v[b_i])))

    in_maps = []
    for c in range(N_CORES):
        b_i, g = divmod(c, HG)
        wo_c = Wo[:, g * EW:(g + 1) * EW].T  # [EW, D]
        wo_c = np.ascontiguousarray(
            wo_c.reshape(HG, DK, D).transpose(1, 0, 2))  # [DK, HG, D]
        in_maps.append({
            "xq": xs[b_i][0], "xk": xs[b_i][1], "xv": xs[b_i][2],
            "wq": aug_w(Wq, bq, g, scale),
            "wk": aug_w(Wk, bk, g),
            "wv": aug_w(Wv, bv, g),
            "wo": wo_c,
        })
    return in_maps


def kernel(q, k, v, Wq, bq, Wk, bk, Wv, bv, Wo, bo):
    q = np.asarray(q, np.float32)
    k = np.asarray(k, np.float32)
    v = np.asarray(v, np.float32)
    in_maps = _prep_inputs(q, k, v,
                           np.asarray(Wq, np.float32), np.asarray(bq, np.float32),
                           np.asarray(Wk, np.float32), np.asarray(bk, np.float32),
                           np.asarray(Wv, np.float32), np.asarray(bv, np.float32),
                           np.asarray(Wo, np.float32), np.asarray(bo, np.float32))
    nc = _get_program()
    res = run_bass_kernel_spmd(nc, in_maps, core_ids=list(range(N_CORES)))
    out = np.zeros((B, S, D), np.float32)
    for c in range(N_CORES):
        b_i = c // HG
        out[b_i] += res.results[c]["outT"].T
    out += np.asarray(bo, np.float32)
    return out


# revision 8
# speedup vs baseline: 3.1203x; 1.0637x over previous
"""Multi-head attention Trainium2 kernel (8 NeuronCores, SPMD).

Problem: B=2, S=2048, D=1024, H=16 heads, d_k=64.
Sharding: 2 batches x 4 head-groups -> 8 cores. Core c handles batch c//4,
heads [4*(c%4), 4*(c%4)+4). Each core computes its 4 heads' Q/K/V
projections, attention, and a partial output projection (row-parallel Wo);
the host sums the 4 partials per batch (the "all-reduce" done on host).

On-device layout is feature-major ("transposed"): activations live as
[d, tokens] so that
  - projections are natural matmuls (lhsT = W.T tiles, rhs = x.T tiles),
  - scores are computed directly as S.T [k_seq, q] (k on partitions),
  - softmax sum over k (partition dim) falls out of the P@V matmul by
    augmenting V with a ones column (row 64 of the PV psum = denominators).
Biases are folded in by augmenting x.T / W.T with a constant-one row
(padded contraction dim 1024 -> 1152 = 9*128). The 1/sqrt(d_k) scale is
folded into Wq/bq on the host. Matmul operands use float32r (full-rate
PE fp32); accumulation stays fp32.

Phase order: K proj, V proj, then per q-tile {Q proj, attention, softmax
normalize, output projection} so DMA/PE/ACT/DVE phases overlap.
"""

import threading
from contextlib import ExitStack

import numpy as np

import concourse.bass as bass
import concourse.tile as tile
from concourse import bacc, mybir
from concourse.bass_utils import run_bass_kernel_spmd

F32 = mybir.dt.float32
F32R = mybir.dt.float32r
AF = mybir.ActivationFunctionType

B = 2
S = 2048
D = 1024
H = 16
DK = 64
N_CORES = 8
HG = 4  # heads per core
EW = HG * DK  # 256 features per core
DP = D + 128  # padded contraction (bias row + zero pad): 9 * 128
DT = DP // 128  # 9 contraction tiles
QT = 4  # q tiles of 512
QTS = S // QT  # 512
KT = S // 128  # 16 k-seq tiles of 128
ET = D // 128  # 8 output-feature tiles


def build_program():
    """Build + compile the (single, SPMD) Bass program. Returns nc."""
    nc = bacc.Bacc("TRN2", target_bir_lowering=False, debug=False,
                   num_devices=N_CORES)

    xq = nc.dram_tensor("xq", [DP, S], F32R, kind="ExternalInput").ap()
    xk = nc.dram_tensor("xk", [DP, S], F32R, kind="ExternalInput").ap()
    xv = nc.dram_tensor("xv", [DP, S], F32R, kind="ExternalInput").ap()
    wq = nc.dram_tensor("wq", [DP, EW], F32R, kind="ExternalInput").ap()
    wk = nc.dram_tensor("wk", [DP, EW], F32R, kind="ExternalInput").ap()
    wv = nc.dram_tensor("wv", [DP, EW], F32R, kind="ExternalInput").ap()
    wo = nc.dram_tensor("wo", [128, 2, D], F32R, kind="ExternalInput").ap()
    outT = nc.dram_tensor("outT", [D, S], F32, kind="ExternalOutput").ap()

    xq_v = xq.rearrange("(dt p) n -> p dt n", p=128)
    xk_v = xk.rearrange("(dt p) n -> p dt n", p=128)
    xv_v = xv.rearrange("(dt p) n -> p dt n", p=128)
    wq_v = wq.rearrange("(dt p) m -> p dt m", p=128)
    wk_v = wk.rearrange("(dt p) m -> p dt m", p=128)
    wv_v = wv.rearrange("(dt p) m -> p dt m", p=128)
    outT_v = outT.rearrange("(et p) n -> p et n", p=128)

    with tile.TileContext(nc) as tc, ExitStack() as ctx:
        persist = ctx.enter_context(tc.tile_pool(name="persist", bufs=1))
        wo_sb = persist.tile([128, 2, D], F32R, tag="wo_sb")
        QTi = persist.tile([128, 2, S], F32R, tag="QT")  # pair-stacked Q.T
        KTi = persist.tile([128, 2, S], F32R, tag="KT")
        VA = persist.tile([128, KT, HG, DK + 1], F32R, tag="VA")
        wpool = ctx.enter_context(tc.tile_pool(name="wts", bufs=1))
        wq_sb = wpool.tile([128, DT, EW], F32R, tag="wq_sb")
        wk_sb = wpool.tile([128, DT, EW], F32R, tag="wk_sb")
        wv_sb = wpool.tile([128, DT, EW], F32R, tag="wv_sb")
        nc.sync.dma_start(wk_sb[:], wk_v[:])
        nc.gpsimd.memset(VA[:, :, :, DK:DK + 1].bitcast(F32), 1.0)

        xpool = ctx.enter_context(tc.tile_pool(name="xin", bufs=2))
        # single PSUM pool, statically 8 banks:
        #   tag "s": [128, 2, 512] (2 banks) x 2 bufs = 4 banks
        #   tag "o": [128, 512]    (1 bank)  x 4 bufs = 4 banks
        psp = ctx.enter_context(tc.tile_pool(name="psp", bufs=1, space="PSUM"))
        ptpool = ctx.enter_context(tc.tile_pool(name="pt", bufs=3))
        oapool = ctx.enter_context(tc.tile_pool(name="oa", bufs=2))
        rpool = ctx.enter_context(tc.tile_pool(name="rr", bufs=1))
        apool = ctx.enter_context(tc.tile_pool(name="atile", bufs=2))
        obuf = ctx.enter_context(tc.tile_pool(name="obuf", bufs=3))

        def otile(name):
            return psp.tile([128, QTS], F32, tag="o", bufs=4, name=name)

        def stile(name):
            return psp.tile([128, 2, QTS], F32, tag="s", bufs=2, name=name)

        # ---- K projection (transposed layout), then V (natural layout) ----
        for qt in range(QT):
            qs = slice(qt * QTS, (qt + 1) * QTS)
            xt = xpool.tile([128, DT, QTS], F32R, tag="xt", name="xkt")
            nc.sync.dma_start(xt[:], xk_v[:, :, qs])
            for p in range(2):
                ps = otile("psk")
                for dt in range(DT):
                    nc.tensor.matmul(
                        ps[:], wk_sb[:, dt, p * 128:(p + 1) * 128],
                        xt[:, dt, :], start=(dt == 0), stop=(dt == DT - 1))
                nc.vector.tensor_copy(KTi[:, p, qs], ps[:])

        nc.sync.dma_start(wq_sb[:], wq_v[:])

        def q_proj(qt):
            qs = slice(qt * QTS, (qt + 1) * QTS)
            xt = xpool.tile([128, DT, QTS], F32R, tag="xt", name="xqt")
            nc.sync.dma_start(xt[:], xq_v[:, :, qs])
            for p in range(2):
                ps = otile("psq")
                for dt in range(DT):
                    nc.tensor.matmul(
                        ps[:], wq_sb[:, dt, p * 128:(p + 1) * 128],
                        xt[:, dt, :], start=(dt == 0), stop=(dt == DT - 1))
                nc.vector.tensor_copy(QTi[:, p, qs], ps[:])

        q_proj(0)
        nc.sync.dma_start(wv_sb[:], wv_v[:])
        nc.sync.dma_start(wo_sb[:], wo[:])

        def v_proj(st):
            ss_ = slice(st * QTS, (st + 1) * QTS)
            xt = xpool.tile([128, DT, QTS], F32R, tag="xt", name="xvt")
            nc.sync.dma_start(xt[:], xv_v[:, :, ss_])
            for ss in range(4):
                kti = st * 4 + ss
                ps = otile("psv")
                for dt in range(DT):
                    nc.tensor.matmul(
                        ps[:, 0:EW], xt[:, dt, ss * 128:(ss + 1) * 128],
                        wv_sb[:, dt, :], start=(dt == 0), stop=(dt == DT - 1))
                nc.vector.tensor_copy(
                    VA[:, kti, :, 0:DK],
                    ps[:, 0:EW].rearrange("p (h d) -> p h d", h=HG))

        def attn_pair_kts(qt, p, po, kts):
            qs = slice(qt * QTS, (qt + 1) * QTS)
            for kt in kts:
                ks = slice(kt * 128, (kt + 1) * 128)
                ps_s = stile("pss")
                for hh in range(2):
                    r0 = 64 * hh
                    nc.tensor.matmul(
                        ps_s[:, hh, :],
                        KTi[r0:r0 + 64, p, ks],
                        QTi[r0:r0 + 64, p, qs],
                        start=True, stop=True)
                pt_t = ptpool.tile([128, 2, QTS], F32R, tag="pt")
                nc.scalar.activation(pt_t[:], ps_s[:], AF.Exp)
                for hh in range(2):
                    h = 2 * p + hh
                    nc.tensor.matmul(
                        po[hh][0:DK + 1, :], VA[:, kt, h, :],
                        pt_t[:, hh, :],
                        start=(kt == 0), stop=(kt == KT - 1))

        def attn_finish_pair(p, po, OA_t):
            for hh in range(2):
                h = 2 * p + hh
                nc.vector.tensor_copy(
                    OA_t[0:DK + 1, h, :], po[hh][0:DK + 1, :])

        def norm_outproj(qt, OA_t):
            qs = slice(qt * QTS, (qt + 1) * QTS)
            r0_t = rpool.tile([1, HG, QTS], F32, tag="r0", name="r0_t")
            rb_t = rpool.tile([128, HG, QTS], F32, tag="rb", name="rb_t")
            nc.sync.dma_start(r0_t[0:1, :, :], OA_t[DK:DK + 1, :, :])
            nc.vector.reciprocal(r0_t[0:1, :, :], r0_t[0:1, :, :])
            nc.gpsimd.partition_broadcast(rb_t[:], r0_t[0:1, :, :])
            # odd heads shifted to partitions 64:128 so the pair shares one
            # K=128 out-proj matmul
            OAs = apool.tile([128, 2, QTS], F32, tag="oas", name="OAs",
                             bufs=1)
            a2s = []
            for p in range(2):
                nc.sync.dma_start(
                    OAs[DK:128, p, :], OA_t[0:DK, 2 * p + 1, :])
            for p in range(2):
                a2 = apool.tile([128, QTS], F32R, tag="at", name=f"a2_{p}")
                nc.vector.tensor_mul(
                    a2[0:DK, :], OA_t[0:DK, 2 * p, :], rb_t[0:DK, 2 * p, :])
                nc.vector.tensor_mul(
                    a2[DK:128, :], OAs[DK:128, p, :],
                    rb_t[DK:128, 2 * p + 1, :])
                a2s.append(a2)
            for et in range(ET):
                ps = otile("pso")
                for p in range(2):
                    nc.tensor.matmul(
                        ps[:], wo_sb[:, p, et * 128:(et + 1) * 128],
                        a2s[p][:], start=(p == 0), stop=(p == 1))
                ot = obuf.tile([128, QTS], F32, tag="ot")
                nc.vector.tensor_copy(ot[:], ps[:])
                nc.sync.dma_start(outT_v[:, et, qs], ot[:])

        # V proj interleaved with attention(qt0, pair0); then steady-state
        # per-qt pipeline with norm+out-proj one qt behind.
        OA_t0 = oapool.tile([128, HG, QTS], F32, tag="oa", name="OA_t0")
        po0 = [otile(f"po0_{i}") for i in range(2)]
        for st in range(QT):
            v_proj(st)
            attn_pair_kts(0, 0, po0, range(4 * st, 4 * st + 4))
        attn_finish_pair(0, po0, OA_t0)

        q_proj(1)
        po1 = [otile(f"po1_{i}") for i in range(2)]
        attn_pair_kts(0, 1, po1, range(KT))
        attn_finish_pair(1, po1, OA_t0)

        pending = (0, OA_t0)
        for qt in range(1, QT):
            if qt + 1 < QT:
                q_proj(qt + 1)
            OA_t = oapool.tile([128, HG, QTS], F32, tag="oa", name="OA_t")
            for p in range(2):
                po = [otile(f"po{i}") for i in range(2)]
                attn_pair_kts(qt, p, po, range(KT))
                attn_finish_pair(p, po, OA_t)
            norm_outproj(*pending)
            pending = (qt, OA_t)
        norm_outproj(*pending)

    nc.compile()
    return nc


_CACHE = {}
_CACHE_LOCK = threading.Lock()


def _get_program():
    with _CACHE_LOCK:
        if "nc" not in _CACHE:
            _CACHE["nc"] = build_program()
        return _CACHE["nc"]


def _prep_inputs(q, k, v, Wq, bq, Wk, bk, Wv, bv, Wo, bo):
    """Build the 8 per-core input maps (all float32 numpy)."""
    scale = 1.0 / np.sqrt(DK)

    def aug_x(x_b):  # [S, D] -> [DP, S]
        out = np.zeros((DP, S), np.float32)
        out[:D] = x_b.T
        out[D] = 1.0
        return np.ascontiguousarray(out)

    def aug_w(W, b, g, sc=1.0):  # rows slice of W -> [DP, EW]
        sl = slice(g * EW, (g + 1) * EW)
        out = np.zeros((DP, EW), np.float32)
        out[:D] = W[sl].T * sc
        out[D] = b[sl] * sc
        return np.ascontiguousarray(out)

    xs = []
    for b_i in range(B):
        xs.append((aug_x(q[b_i]), aug_x(k[b_i]), aug_x(v[b_i])))

    in_maps = []
    for c in range(N_CORES):
        b_i, g = divmod(c, HG)
        wo_c = Wo[:, g * EW:(g + 1) * EW].T  # [EW, D] = [(2 pairs x 2 x DK), D]
        wo_c = np.ascontiguousarray(
            wo_c.reshape(2, 128, D).transpose(1, 0, 2))  # [128, 2, D]
        in_maps.append({
            "xq": xs[b_i][0], "xk": xs[b_i][1], "xv": xs[b_i][2],
            "wq": aug_w(Wq, bq, g, scale),
            "wk": aug_w(Wk, bk, g),
            "wv": aug_w(Wv, bv, g),
            "wo": wo_c,
        })
    return in_maps


def kernel(q, k, v, Wq, bq, Wk, bk, Wv, bv, Wo, bo):
    q = np.asarray(q, np.float32)
    k = np.asarray(k, np.float32)
    v = np.asarray(v, np.float32)
    in_maps = _prep_inputs(q, k, v,
                           np.asarray(Wq, np.float32), np.asarray(bq, np.float32),
                           np.asarray(Wk, np.float32), np.asarray(bk, np.float32),
                           np.asarray(Wv, np.float32), np.asarray(bv, np.float32),
                           np.asarray(Wo, np.float32), np.asarray(bo, np.float32))
    nc = _get_program()
    res = run_bass_kernel_spmd(nc, in_maps, core_ids=list(range(N_CORES)))
    out = np.zeros((B, S, D), np.float32)
    for c in range(N_CORES):
        b_i = c // HG
        out[b_i] += res.results[c]["outT"].T
    out += np.asarray(bo, np.float32)
    return out
